# revision 3
# baseline (speedup 1.0000x reference)
"""Trainium2 Bass kernel for nn_EncoderStack (dense transformer encoder layer).

Strategy (8 NeuronCores, single NEFF launch):
  Phase 1 (head-parallel): each core owns 2 of 16 heads. Computes q/k/v
  projections (feature-major via pre-transposed x), scores^T = k q^T per
  head with softmax over the query axis (free dim), folds the softmax
  denominator into v, accumulates o^T = v'^T @ attn^T in PSUM.
  AllToAll: cores exchange per-head o^T blocks so each core ends up with
  the full 1024 attention features for its 512-token slice (1 MiB bf16
  per core on the wire instead of 16 MiB fp32 all-reduce partials).
  Phase 2 (token-parallel): Wo projection + residual + norm, FFN
  (relu(x@W1+b1)@W2+b2) + residual + norm on the local 512 tokens.
  Host concatenates the per-core token slices.

All matmuls run in bf16 with fp32 PSUM accumulation; residuals and
normalization statistics stay fp32.
"""

import numpy as np

B, T, D = 2, 2048, 1024
H, DK, DV = 16, 64, 64
FF = 4096
N_CORES = 8
P = 128
TOK = B * T          # 4096 tokens
TPC = TOK // N_CORES  # 512 tokens per core
HPC = H // N_CORES   # 2 heads per core
KT = D // P          # 8 k-tiles over D
FT = FF // P         # 32 f-tiles over FF
ST = T // P          # 16 s-tiles per batch
TT = TPC // P        # 4 token-tiles per core

_CACHE = {}


def _build():
    import concourse.bass as bass
    import concourse.bacc as bacc
    import concourse.mybir as mybir
    from concourse import tile
    from concourse.masks import make_identity

    f32 = mybir.dt.float32
    bf16 = mybir.dt.bfloat16
    AX = mybir.AxisListType
    AF = mybir.ActivationFunctionType
    ALU = mybir.AluOpType

    nc = bacc.Bacc("TRN2", target_bir_lowering=False, debug=False,
                   enable_asserts=True, num_devices=N_CORES)

    xt_d = nc.dram_tensor("xt", [KT, P, TOK], bf16, kind="ExternalInput")
    xres_d = nc.dram_tensor("xres", [TPC, D], f32, kind="ExternalInput")
    wqkv_d = nc.dram_tensor("wqkv", [KT, P, 384], bf16, kind="ExternalInput")
    wo_d = nc.dram_tensor("wo", [KT, P, D], bf16, kind="ExternalInput")
    w1_d = nc.dram_tensor("w1", [FT, KT, P, P], bf16, kind="ExternalInput")
    b1_d = nc.dram_tensor("b1", [P, FT], f32, kind="ExternalInput")
    w2_d = nc.dram_tensor("w2", [FT, P, D], bf16, kind="ExternalInput")
    b2_d = nc.dram_tensor("b2", [P, D], f32, kind="ExternalInput")
    out_d = nc.dram_tensor("out", [TPC, D], f32, kind="ExternalOutput")

    xres_r = xres_d.ap().rearrange("(a p) d -> a p d", p=P)
    out_r = out_d.ap().rearrange("(a p) d -> a p d", p=P)

    with tile.TileContext(nc) as tc:
        with tc.tile_pool(name="wts", bufs=1) as wts, \
             tc.tile_pool(name="small", bufs=6) as small, \
             tc.tile_pool(name="o1", bufs=1) as o1p, \
             tc.tile_pool(name="ps", bufs=2, space="PSUM") as psp, \
             tc.tile_pool(name="dram", bufs=1, space="DRAM") as dram:

            ident = wts.tile([P, P], f32)
            make_identity(nc, ident)
            def dma3(dst2d, src_ap, a):
                nc.sync.dma_start(
                    dst2d.rearrange("p (a m) -> p a m", a=a),
                    src_ap.rearrange("a p m -> p a m"))

            wqkv_sb = wts.tile([P, KT * 384], bf16)
            dma3(wqkv_sb[:], wqkv_d.ap(), KT)
            wo_sb = wts.tile([P, KT * D], bf16)
            dma3(wo_sb[:], wo_d.ap(), KT)
            b1_sb = wts.tile([P, FT], f32)
            nc.sync.dma_start(b1_sb[:], b1_d.ap())
            b2_sb = wts.tile([P, D], f32)
            nc.sync.dma_start(b2_sb[:], b2_d.ap())

            out1_sb = o1p.tile([P, TT * D], f32)     # norm1 output, fp32 residual
            out1T_sb = o1p.tile([P, KT * TPC], bf16)  # transposed for FFN
            oall_sb = o1p.tile([P, KT * TPC], bf16)   # gathered attention features

            a2a_in = dram.tile([N_CORES, P, TPC], bf16)
            a2a_out = dram.tile([N_CORES, P, TPC], bf16)

            # ---------------- Phase 1: attention (2 heads, all tokens) ----
            with tc.tile_pool(name="p1", bufs=1) as p1, \
                 tc.tile_pool(name="p1b", bufs=2) as p1b, \
                 tc.tile_pool(name="atp", bufs=4) as atp:
                for b in range(B):
                    xt_b = p1.tile([P, KT * T], bf16, tag="xt", bufs=2)
                    dma3(xt_b[:], xt_d.ap()[:, :, b * T:(b + 1) * T], KT)

                    q_sb = p1b.tile([P, T], bf16, tag="q")
                    k_sb = p1b.tile([P, T], bf16, tag="k")
                    v_sb = p1b.tile([P, T], bf16, tag="v")
                    # q^T / k^T (dk-packed 2 heads on partitions), feature-major
                    for dst, wofs in ((q_sb, 0), (k_sb, P)):
                        for half in range(2):
                            pt = psp.tile([P, 1024], f32, tag="mm")
                            for kt in range(KT):
                                for c in range(2):
                                    ofs = half * 1024 + c * 512
                                    nc.tensor.matmul(
                                        pt[:, c * 512:(c + 1) * 512],
                                        wqkv_sb[:, kt * 384 + wofs: kt * 384 + wofs + P],
                                        xt_b[:, kt * T + ofs: kt * T + ofs + 512],
                                        start=(kt == 0), stop=(kt == KT - 1))
                            nc.vector.tensor_copy(dst[:, half * 1024: half * 1024 + 1024], pt[:])
                    # v token-major (dv-packed 2 heads on free dim)
                    for st in range(ST):
                        pv = psp.tile([P, P], f32, tag="mm")
                        for kt in range(KT):
                            nc.tensor.matmul(
                                pv[:],
                                xt_b[:, kt * T + st * P: kt * T + (st + 1) * P],
                                wqkv_sb[:, kt * 384 + 256: kt * 384 + 384],
                                start=(kt == 0), stop=(kt == KT - 1))
                        nc.vector.tensor_copy(v_sb[:, st * P:(st + 1) * P], pv[:])

                    for h in range(HPC):
                        hofs = 64 * h
                        o_ps = psp.tile([64, T], f32, tag="o", bufs=1)
                        for st in range(ST):
                            at_tiles = []
                            zp = small.tile([P, 2], f32, tag="zp")
                            for half in range(2):
                                sc = psp.tile([P, 1024], f32, tag="mm")
                                for c in range(2):
                                    ofs = half * 1024 + c * 512
                                    nc.tensor.matmul(
                                        sc[:, c * 512:(c + 1) * 512],
                                        k_sb[hofs:hofs + 64, st * P:(st + 1) * P],
                                        q_sb[hofs:hofs + 64, ofs: ofs + 512],
                                        start=True, stop=True)
                                at = atp.tile([P, 1024], bf16, tag="at")
                                nc.scalar.activation(at[:], sc[:], AF.Exp,
                                                     scale=0.125,
                                                     accum_out=zp[:, half:half + 1])
                                at_tiles.append(at)
                            zs = small.tile([P, 1], f32, tag="zs")
                            nc.vector.tensor_add(zs[:], zp[:, 0:1], zp[:, 1:2])
                            zi = small.tile([P, 1], f32, tag="zi")
                            nc.vector.reciprocal(zi[:], zs[:])
                            vp = small.tile([P, 64], bf16, tag="vp")
                            nc.vector.tensor_scalar_mul(
                                vp[:], v_sb[:, st * P + hofs: st * P + hofs + 64], zi[:])
                            for half in range(2):
                                for c in range(2):
                                    ofs = half * 1024 + c * 512
                                    nc.tensor.matmul(
                                        o_ps[:, ofs: ofs + 512],
                                        vp[:],
                                        at_tiles[half][:, c * 512:(c + 1) * 512],
                                        start=(st == 0), stop=(st == ST - 1))
                        ot = p1b.tile([64, T], bf16, tag="ot")
                        nc.vector.tensor_copy(ot[:], o_ps[:])
                        for k4 in range(4):
                            nc.sync.dma_start(
                                a2a_in[4 * b + k4, hofs:hofs + 64, :],
                                ot[:, k4 * TPC:(k4 + 1) * TPC])

            nc.gpsimd.collective_compute(
                "AllToAll", ALU.bypass,
                replica_groups=[list(range(N_CORES))],
                ins=[a2a_in.opt()], outs=[a2a_out.opt()])

            # ---------------- Phase 2: Wo + norm + FFN on 512 tokens ------
            with tc.tile_pool(name="p2", bufs=2) as p2:
                for kt in range(KT):
                    nc.sync.dma_start(oall_sb[:, kt * TPC:(kt + 1) * TPC], a2a_out[kt])
                w2_sb = p2.tile([P, FT * D], bf16, tag="w2", bufs=1)
                dma3(w2_sb[:], w2_d.ap(), FT)
                h1T_sb = p2.tile([P, FT * TPC], bf16, tag="h1t", bufs=1)

                def norm_rows(y_ap, out_ap):
                    ssum = small.tile([P, 1], f32, tag="st1")
                    nc.vector.reduce_sum(ssum[:], y_ap, axis=AX.X)
                    mean = small.tile([P, 1], f32, tag="st2")
                    nc.scalar.mul(mean[:], ssum[:], 1.0 / D)
                    negmean = small.tile([P, 1], f32, tag="st3")
                    nc.scalar.mul(negmean[:], ssum[:], -1.0 / D)
                    sq = p2.tile([P, D], bf16, tag="sq")
                    ssq = small.tile([P, 1], f32, tag="st4")
                    nc.scalar.activation(sq[:], y_ap, AF.Square,
                                         bias=negmean[:], accum_out=ssq[:])
                    sdev = small.tile([P, 1], f32, tag="st5")
                    nc.scalar.activation(sdev[:], ssq[:], AF.Sqrt, scale=1.0 / (D - 1))
                    istd = small.tile([P, 1], f32, tag="st6")
                    nc.vector.reciprocal(istd[:], sdev[:])
                    nc.vector.tensor_scalar(out_ap, y_ap, mean[:], istd[:],
                                            op0=ALU.subtract, op1=ALU.mult)

                # Wo + residual + norm1, then transpose out1 for the FFN
                for tt in range(TT):
                    pw = psp.tile([P, D], f32, tag="mm")
                    for kt in range(KT):
                        for c in range(2):
                            nc.tensor.matmul(
                                pw[:, c * 512:(c + 1) * 512],
                                oall_sb[:, kt * TPC + tt * P: kt * TPC + (tt + 1) * P],
                                wo_sb[:, kt * D + c * 512: kt * D + (c + 1) * 512],
                                start=(kt == 0), stop=(kt == KT - 1))
                    xr = p2.tile([P, D], f32, tag="xr")
                    nc.sync.dma_start(xr[:], xres_r[tt])
                    y = p2.tile([P, D], f32, tag="y")
                    nc.vector.tensor_add(y[:], pw[:], xr[:])
                    norm_rows(y[:], out1_sb[:, tt * D:(tt + 1) * D])
                    for kt in range(KT):
                        ptr = psp.tile([P, P], f32, tag="o", bufs=1)
                        nc.tensor.transpose(
                            ptr[:], out1_sb[:, tt * D + kt * P: tt * D + (kt + 1) * P],
                            ident[:])
                        nc.vector.tensor_copy(
                            out1T_sb[:, kt * TPC + tt * P: kt * TPC + (tt + 1) * P],
                            ptr[:])

                # h1^T = relu(W1^T @ out1^T + b1), feature-major
                for ft in range(FT):
                    w1s = p2.tile([P, KT * P], bf16, tag="w1s")
                    dma3(w1s[:], w1_d.ap()[ft], KT)
                    ph = psp.tile([P, TPC], f32, tag="mm")
                    for kt in range(KT):
                        nc.tensor.matmul(
                            ph[:],
                            w1s[:, kt * P:(kt + 1) * P],
                            out1T_sb[:, kt * TPC:(kt + 1) * TPC],
                            start=(kt == 0), stop=(kt == KT - 1))
                    nc.scalar.activation(h1T_sb[:, ft * TPC:(ft + 1) * TPC], ph[:],
                                         AF.Relu, bias=b1_sb[:, ft:ft + 1])

                # ff = h1 @ W2 + b2, + residual + norm2, write out
                for tt in range(TT):
                    pf = psp.tile([P, D], f32, tag="mm")
                    for ft in range(FT):
                        for c in range(2):
                            nc.tensor.matmul(
                                pf[:, c * 512:(c + 1) * 512],
                                h1T_sb[:, ft * TPC + tt * P: ft * TPC + (tt + 1) * P],
                                w2_sb[:, ft * D + c * 512: ft * D + (c + 1) * 512],
                                start=(ft == 0), stop=(ft == FT - 1))
                    y2 = p2.tile([P, D], f32, tag="y")
                    nc.vector.tensor_add(y2[:], pf[:], out1_sb[:, tt * D:(tt + 1) * D])
                    nc.vector.tensor_add(y2[:], y2[:], b2_sb[:])
                    o2 = p2.tile([P, D], f32, tag="o2")
                    norm_rows(y2[:], o2[:])
                    nc.sync.dma_start(out_r[tt], o2[:])

    nc.compile()
    return nc


def _get_nc():
    if "nc" not in _CACHE:
        _CACHE["nc"] = _build()
    return _CACHE["nc"]


def _prep_inputs(x, Wq, Wk, Wv, Wo, W1, b1, W2, b2):
    import ml_dtypes
    bf = ml_dtypes.bfloat16
    x = np.asarray(x, np.float32)
    x2 = np.ascontiguousarray(x.reshape(TOK, D))
    xt = np.ascontiguousarray(x2.T).astype(bf).reshape(KT, P, TOK)
    wo8 = np.ascontiguousarray(np.asarray(Wo, np.float32).astype(bf).reshape(KT, P, D))
    w1t = np.ascontiguousarray(
        np.asarray(W1, np.float32).astype(bf).reshape(KT, P, FT, P).transpose(2, 0, 1, 3))
    w2t = np.ascontiguousarray(np.asarray(W2, np.float32).astype(bf).reshape(FT, P, D))
    b1t = np.ascontiguousarray(np.asarray(b1, np.float32).reshape(FT, P).T)
    b2b = np.ascontiguousarray(
        np.broadcast_to(np.asarray(b2, np.float32).reshape(1, D), (P, D)))
    Wq = np.asarray(Wq, np.float32)
    Wk = np.asarray(Wk, np.float32)
    Wv = np.asarray(Wv, np.float32)
    in_maps = []
    for c in range(N_CORES):
        h0 = HPC * c
        wqkv = np.concatenate(
            [Wq[h0], Wq[h0 + 1], Wk[h0], Wk[h0 + 1], Wv[h0], Wv[h0 + 1]],
            axis=1).astype(bf)  # [D, 384]: q-pack | k-pack | v-pack
        wqkv = np.ascontiguousarray(wqkv.reshape(KT, P, 384))
        xres = np.ascontiguousarray(x2[c * TPC:(c + 1) * TPC])
        in_maps.append({
            "xt": xt, "xres": xres, "wqkv": wqkv, "wo": wo8,
            "w1": w1t, "b1": b1t, "w2": w2t, "b2": b2b,
        })
    return in_maps


def kernel(x, Wq, Wk, Wv, Wo, W1, b1, W2, b2):
    from concourse.bass_utils import run_bass_kernel_spmd
    nc = _get_nc()
    in_maps = _prep_inputs(x, Wq, Wk, Wv, Wo, W1, b1, W2, b2)
    res = run_bass_kernel_spmd(nc, in_maps, core_ids=list(range(N_CORES)))
    out = np.concatenate([np.asarray(res.results[c]["out"], np.float32)
                          for c in range(N_CORES)], axis=0)
    return out.reshape(B, T, D)
